# revision 2
# baseline (speedup 1.0000x reference)
"""Trainium2 Bass kernel v5 for NeuralSumProductModel (LDPC sum-product).

Contract: kernel(llr [512,8192] f32, var_index [24576] i32, chk_index
[24576] i32) -> [5, 512, 8192] f32, batch-sharded 8 ways (BC=64/core).

v5: all-gather dataflow (dma_scatter_add drains at ~100ns/desc DRAM RMW
— unusable), fp16 staging padded to 256B rows so every gather desc is
256B (the legal minimum), all contiguous/strided writes on HWDGE (no
gpsimd cost):
  - OUTR[2] fp16 [N_VAR, 128]: out rows (64 payload + 64 pad), ping-pong.
  - EXTR fp16 [E, 128]: check-major ext rows, written strided by HWDGE.
  - check unit (3072 edges): msg-gather out[var(e)] (1 desc/edge, 256B)
    -> msg = g - ext_prev (1 DVE op); Tanh -> t(f32); prefix/suffix
    products -> loo (signs ride along); clamp +-(1-1e-7); ext =
    Ln(1+loo) - Ln(1-loo); strided HWDGE write to EXTR.
  - var phase (4 chunks): var-gather each var's 3 ext rows (1 desc each)
    -> out = x + s0+s1+s2 -> strided HWDGE write to OUTR[next].
  - out_d: ACT transpose-copy + write, deferred into the next iteration.
  - iteration 0 needs no special case: OUTR[0] pre-initialized with x,
    ext_prev = 0 (memset).
  - Tanh/Ln activation tables batched pairwise (8 loads/iter).
"""

import os
import sys

import numpy as np

for _p in ("/opt/trn_rl_repo", "/root/.axon_site/_ro/trn_rl_repo"):
    if os.path.isdir(_p) and _p not in sys.path:
        sys.path.insert(0, _p)

N_VAR, N_CHK, DV, DC = 8192, 4096, 3, 6
E = N_VAR * DV                  # 24576
BATCH, N_ITER, N_CORES = 512, 5, 8
BC = BATCH // N_CORES           # 64 batch rows per core
NU = 8                          # check units per iteration
UE = E // NU                    # 3072 edges per unit
UJ = UE // 128                  # 24 gathered blocks per unit
CU = UJ * BC                    # 1536 payload cols per unit
CTU = N_CHK // 128 // NU        # 4 check tiles per unit
GCH = 1024                      # idxs per SWDGE call (ring cap)
VROW = 128                      # staging row width (64 payload + 64 pad)
NVC = 4                         # var-phase chunks
VCH = N_VAR // 128 // NVC       # 16 vars per partition per chunk

EPS = 1e-12
CLIP = float(np.float32(1.0) - np.float32(1e-7))

_CACHE = {}
_LAST_RESULTS = None


def _wrap(stream):
    """[n] int -> wrapped [128, n//16] int16 (16-partition wrap, x8 cores)."""
    st = np.asarray(stream, np.int16)
    n = st.shape[0]
    assert n % 16 == 0
    core = st.reshape(n // 16, 16).T
    return np.tile(core, (8, 1))


def _build_indices(vi, ci):
    """Host-side graph preprocessing.

    Check-major slot j = u*3072 + l*128 + p  (l = ct_loc*6 + e) maps to
    check c = (4u + l//6)*128 + p, edge e = l%6 of the by-check-sorted
    edge list. EXTR row == slot j. Var-gather output row i = n*128 + p
    (n global) covers (vt, s) = (n//3, n%3), var v = p*64 + vt.
    """
    order = np.argsort(ci, kind="stable")
    cm_var = vi[order].astype(np.int64)

    u = np.arange(NU)[:, None, None]
    l = np.arange(UJ)[None, :, None]
    p = np.arange(128)[None, None, :]
    c = (CTU * u + l // DC) * 128 + p
    cm_idx = c * DC + (l % DC)
    v_slot = cm_var[cm_idx].reshape(E)           # var id per slot j

    # j-positions of each var's 3 edges (sorted by j): pos[v*3+s]
    pos = np.argsort(v_slot, kind="stable")      # [E]

    n = np.arange(N_VAR // 128 * DV)[:, None]    # 192 rows per partition
    p2 = np.arange(128)[None, :]
    v = p2 * (N_VAR // 128) + n // DV
    vidx = pos[v * DV + n % DV]                  # [192, 128]
    vidx = vidx.reshape(-1)                      # i = n*128 + p order

    return _wrap(v_slot), _wrap(vidx), v_slot


def _build_bass():
    import concourse.tile as tile
    from concourse import bacc, mybir
    from contextlib import ExitStack

    dt = mybir.dt
    F32, F16, I16 = dt.float32, dt.float16, dt.int16
    ALU = mybir.AluOpType
    ACT = mybir.ActivationFunctionType

    nc = bacc.Bacc("TRN2", target_bir_lowering=False, debug=False,
                   num_swdge_queues=4)

    gidx_d = nc.dram_tensor("gidx", [128, E // 16], I16,
                            kind="ExternalInput").ap()
    vidx_d = nc.dram_tensor("vidx", [128, E // 16], I16,
                            kind="ExternalInput").ap()
    xv_d = nc.dram_tensor("xv", [N_VAR, BC], F16, kind="ExternalInput").ap()
    xg_d = nc.dram_tensor("xg", [E, BC], F16, kind="ExternalInput").ap()
    out_d = nc.dram_tensor("out", [N_ITER, BC, N_VAR], F32,
                           kind="ExternalOutput").ap()
    outr = [nc.dram_tensor(f"outr{i}", [N_VAR, VROW], F16,
                           kind="Internal").ap() for i in range(2)]
    extr = nc.dram_tensor("extr", [E, VROW], F16, kind="Internal").ap()

    with tile.TileContext(nc) as tc, ExitStack() as ctx:
        big = ctx.enter_context(tc.tile_pool(name="big", bufs=1))

        G = [big.tile([128, UJ * VROW], F16, tag=f"g{i}", name=f"G{i}")
             for i in range(2)]                        # [p,24*128] 6KB each
        GV = [big.tile([128, VCH * DV * VROW], F16, tag=f"gv{i}",
                       name=f"GV{i}") for i in range(3)]  # [p,48*128] 12KB
        xvt = big.tile([128, N_VAR // 2], F16, tag="xv")   # 8KB
        xgt = big.tile([128, E * BC // 128], F16, tag="xg")  # 24KB
        exts = big.tile([128, E * BC // 128], F16, tag="exts")  # 24KB
        T = [big.tile([128, CU], F32, tag=f"t{i}", name=f"T{i}")
             for i in range(2)]
        LOO = [big.tile([128, CU], F32, tag=f"loo{i}", name=f"LOO{i}")
               for i in range(2)]
        MA = [big.tile([128, CU], F16, tag=f"ma{i}", name=f"MA{i}")
              for i in range(2)]
        MB = [big.tile([128, CU], F16, tag=f"mb{i}", name=f"MB{i}")
              for i in range(2)]
        sA = big.tile([128, CTU * BC], F32, tag="sa")      # 1KB
        sB = big.tile([128, CTU * BC], F32, tag="sb")
        vtA = big.tile([128, VCH * BC], F16, tag="vta")    # 2KB
        vtB = big.tile([128, VCH * BC], F16, tag="vtb")    # 2KB
        OV = [big.tile([128, N_VAR // 2], F16, tag=f"ov{i}", name=f"OV{i}")
              for i in range(2)]                           # 8KB each
        ob = big.tile([128, N_VAR // 2], F32, tag="ob")    # 16KB
        bclip = big.tile([128, 1], F32, tag="bclip")
        beps = big.tile([128, 1], F32, tag="beps")
        gidx_t = big.tile([128, E // 16], I16, tag="gidx")
        vidx_t = big.tile([128, E // 16], I16, tag="vidx")

        nc.sync.dma_start(gidx_t[:], gidx_d[:])
        nc.sync.dma_start(vidx_t[:], vidx_d[:])
        nc.sync.dma_start(xvt[:].rearrange("p (n k) -> p n k", k=BC),
                          xv_d[:, :].rearrange("(p n) k -> p n k", p=128))
        nc.sync.dma_start(xgt[:].rearrange("p (n k) -> p n k", k=BC),
                          xg_d[:, :].rearrange("(n p) k -> p n k", p=128))
        nc.vector.memset(bclip[:], CLIP)
        nc.vector.memset(beps[:], 1.0 - CLIP)
        qc = [0]

        def nextq():
            qc[0] += 1
            return qc[0] % 4

        # OUTR[0] := x rows (strided: 64 payload cols of each 128-col row)
        outr_s = [o[:, :].rearrange("(p n) k -> p n k", p=128)
                  for o in outr]                       # [128, 64, 128]
        extr_s = extr[:, :].rearrange("(n p) k -> p n k", p=128)
        nc.sync.dma_start(outr_s[0][:, :, 0:BC],
                          xvt[:].rearrange("p (n k) -> p n k", k=BC))

        def gathers(it, u):
            src = outr[it % 2][:, :]
            for c in range(3):
                g0 = u * UE + c * GCH
                dsl = G[u % 2][:, c * 8 * VROW:(c + 1) * 8 * VROW]
                nc.gpsimd.dma_gather(
                    dsl.rearrange("p (n k) -> p n k", k=VROW), src,
                    gidx_t[:, g0 // 16:(g0 + GCH) // 16],
                    num_idxs=GCH, num_idxs_reg=GCH, elem_size=VROW,
                    queue_num=nextq())

        def msg(u):
            gv = G[u % 2][:].rearrange("p (n k) -> p n k", k=VROW)
            mbv = MB[u % 2][:].rearrange("p (n k) -> p n k", k=BC)
            epre = exts[:, u * CU:(u + 1) * CU].rearrange(
                "p (n k) -> p n k", k=BC)
            nc.vector.tensor_tensor(mbv, gv[:, :, 0:BC], epre,
                                    op=ALU.subtract)
            return MB[u % 2]

        def products(u):
            t6 = T[u % 2][:].rearrange("p (ct e b) -> p ct e b", ct=CTU, e=DC)
            l6 = LOO[u % 2][:].rearrange("p (ct e b) -> p ct e b",
                                         ct=CTU, e=DC)
            t = [t6[:, :, e, :] for e in range(DC)]
            l = [l6[:, :, e, :] for e in range(DC)]
            a = sA[:].rearrange("p (ct b) -> p ct b", ct=CTU)
            b = sB[:].rearrange("p (ct b) -> p ct b", ct=CTU)
            M = ALU.mult
            nc.vector.tensor_tensor(l[1], t[0], t[1], op=M)   # pre2
            nc.vector.tensor_tensor(l[2], l[1], t[2], op=M)   # pre3
            nc.vector.tensor_tensor(l[3], l[2], t[3], op=M)   # pre4
            nc.vector.tensor_tensor(l[5], l[3], t[4], op=M)   # pre5
            nc.vector.tensor_tensor(l[4], l[3], t[5], op=M)   # pre4*suf5
            nc.vector.tensor_tensor(a, t[4], t[5], op=M)      # suf4
            nc.vector.tensor_tensor(l[3], l[2], a, op=M)      # pre3*suf4
            nc.vector.tensor_tensor(b, t[3], a, op=M)         # suf3
            nc.vector.tensor_tensor(l[2], l[1], b, op=M)      # pre2*suf3
            nc.vector.tensor_tensor(a, t[2], b, op=M)         # suf2
            nc.vector.tensor_tensor(l[1], t[0], a, op=M)      # pre1*suf2
            nc.vector.tensor_tensor(l[0], t[1], a, op=M)      # suf1

        def clamp_ln_ext(it, u):
            nc.vector.tensor_scalar(T[u % 2][:], LOO[u % 2][:], CLIP, -CLIP,
                                    op0=ALU.min, op1=ALU.max)
            nc.scalar.activation(MA[u % 2][:], T[u % 2][:], ACT.Ln,
                                 bias=1.0, scale=1.0)
            nc.scalar.activation(MB[u % 2][:], T[u % 2][:], ACT.Ln,
                                 bias=1.0, scale=-1.0)
            nc.vector.tensor_tensor(exts[:, u * CU:(u + 1) * CU],
                                    MA[u % 2][:], MB[u % 2][:],
                                    op=ALU.subtract)
            # strided HWDGE write of this unit's ext rows to EXTR
            nc.sync.dma_start(
                extr_s[:, u * UJ:(u + 1) * UJ, 0:BC],
                exts[:, u * CU:(u + 1) * CU].rearrange(
                    "p (n k) -> p n k", k=BC))

        def var_gathers(c):
            for q in range(6):
                g0 = c * DV * VCH * 128 + q * GCH
                dsl = GV[c % 3][:, q * 8 * VROW:(q + 1) * 8 * VROW]
                nc.gpsimd.dma_gather(
                    dsl.rearrange("p (n k) -> p n k", k=VROW),
                    extr[:, :],
                    vidx_t[:, g0 // 16:(g0 + GCH) // 16],
                    num_idxs=GCH, num_idxs_reg=GCH, elem_size=VROW,
                    queue_num=nextq())

        def var_sums(it, c):
            g3 = GV[c % 3][:].rearrange("p (vt s k) -> p vt s k",
                                        s=DV, k=VROW)
            s0, s1, s2 = (g3[:, :, s, 0:BC] for s in range(DV))
            w = VCH * BC
            va = vtA[:].rearrange("p (n k) -> p n k", k=BC)
            vb = vtB[:].rearrange("p (n k) -> p n k", k=BC)
            ovw = OV[it % 2][:, c * w:(c + 1) * w].rearrange(
                "p (n k) -> p n k", k=BC)
            xvw = xvt[:, c * w:(c + 1) * w].rearrange("p (n k) -> p n k", k=BC)
            nc.vector.tensor_tensor(va, s0, s1, op=ALU.add)
            nc.vector.tensor_tensor(vb, va, s2, op=ALU.add)
            nc.vector.tensor_tensor(ovw, vb, xvw, op=ALU.add)
            # strided write of this chunk's out rows to OUTR[next]
            if it + 1 < N_ITER:
                nc.sync.dma_start(
                    outr_s[(it + 1) % 2][:, c * VCH:(c + 1) * VCH, 0:BC], ovw)

        def out_finish(src_it):
            nc.scalar.activation(
                ob[:].rearrange("p (b vt) -> p b vt", b=BC),
                OV[src_it % 2][:].rearrange("p (vt b) -> p b vt",
                                            vt=N_VAR // 128),
                ACT.Copy)
            nc.sync.dma_start(
                out_d[src_it].rearrange("b (p vt) -> p b vt", p=128),
                ob[:].rearrange("p (b vt) -> p b vt", b=BC))

        for it in range(N_ITER):
            if it > 0:
                gathers(it, 0)
                gathers(it, 1)
            for k in range(NU // 2):
                a, b2 = 2 * k, 2 * k + 1
                src_a = msg(a)[:] if it > 0 \
                    else xgt[:, a * CU:(a + 1) * CU]
                nc.scalar.activation(T[a % 2][:], src_a, ACT.Tanh, scale=0.5)
                if it > 0 and 2 * k + 2 < NU:
                    gathers(it, 2 * k + 2)
                src_b = msg(b2)[:] if it > 0 \
                    else xgt[:, b2 * CU:(b2 + 1) * CU]
                nc.scalar.activation(T[b2 % 2][:], src_b, ACT.Tanh, scale=0.5)
                if it > 0 and 2 * k + 3 < NU:
                    gathers(it, 2 * k + 3)
                products(a)
                clamp_ln_ext(it, a)
                products(b2)
                clamp_ln_ext(it, b2)
                if k == 0 and it > 0:
                    out_finish(it - 1)
            # var phase: gather each var's 3 ext rows, out = x + sums
            var_gathers(0)
            var_gathers(1)
            var_gathers(2)
            var_sums(it, 0)
            var_gathers(3)
            var_sums(it, 1)
            var_sums(it, 2)
            var_sums(it, 3)
        out_finish(N_ITER - 1)

    nc.compile()
    return nc


def _numpy_fallback(llr, vi, ci):
    x = llr.T.astype(np.float32)
    scattered = x[vi]
    ext = np.zeros_like(scattered)
    outs = []
    for _ in range(N_ITER):
        vsum = np.zeros((N_VAR, x.shape[1]), np.float32)
        np.add.at(vsum, vi, ext)
        msg = (vsum[vi] - ext) + scattered
        t = np.tanh(msg * 0.5)
        la = np.log(np.abs(t) + EPS)
        sg = np.sign(t)
        cs = np.zeros((N_CHK, x.shape[1]), np.float32)
        np.add.at(cs, ci, la)
        cpr = np.ones((N_CHK, x.shape[1]), np.float32)
        np.multiply.at(cpr, ci, sg)
        loo = np.exp(cs[ci] - la) * (cpr[ci] * sg)
        loo = np.clip(loo, -CLIP, CLIP)
        ext = 2.0 * np.arctanh(loo)
        vs2 = np.zeros((N_VAR, x.shape[1]), np.float32)
        np.add.at(vs2, vi, ext)
        outs.append((vs2 + x).T)
    return np.stack(outs)


def kernel(llr, var_index, chk_index):
    llr = np.asarray(llr, np.float32)
    vi = np.asarray(var_index, np.int64).ravel()
    ci = np.asarray(chk_index, np.int64).ravel()
    assert llr.shape == (BATCH, N_VAR) and vi.shape == (E,) and ci.shape == (E,)

    regular = (np.array_equal(np.bincount(vi, minlength=N_VAR),
                              np.full(N_VAR, DV))
               and np.array_equal(np.bincount(ci, minlength=N_CHK),
                                  np.full(N_CHK, DC)))
    if not regular:
        return _numpy_fallback(llr, vi, ci).astype(np.float32)

    key = ("v5", hash(vi.tobytes()), hash(ci.tobytes()))
    if key not in _CACHE:
        gidx, vidx, v_slot = _build_indices(vi, ci)
        nc = _build_bass()
        _CACHE[key] = (nc, gidx, vidx, v_slot)
    nc, gidx, vidx, v_slot = _CACHE[key]

    from concourse.bass_utils import run_bass_kernel_spmd
    llr16 = llr.astype(np.float16)
    in_maps = []
    for c in range(N_CORES):
        xc = llr16[c * BC:(c + 1) * BC, :]
        m = {
            "gidx": np.ascontiguousarray(gidx),
            "vidx": np.ascontiguousarray(vidx),
            "xv": np.ascontiguousarray(xc.T),
            "xg": np.ascontiguousarray(xc.T[v_slot]),
        }
        in_maps.append(m)
    trace = os.environ.get("BASS_KERNEL_TRACE", "0") == "1"
    res = run_bass_kernel_spmd(nc, in_maps, list(range(N_CORES)), trace=trace)
    global _LAST_RESULTS
    _LAST_RESULTS = res
    out = np.concatenate([res.results[c]["out"] for c in range(N_CORES)],
                         axis=1)
    return np.ascontiguousarray(out, dtype=np.float32)


if __name__ == "__main__":
    sys.path.insert(0, os.path.dirname(os.path.abspath(__file__)))
    import reference
    inputs = {k: np.asarray(v) for k, v in reference.setup_inputs().items()}
    exp = _numpy_fallback(np.asarray(inputs["llr"], np.float32),
                          np.asarray(inputs["var_index"], np.int64),
                          np.asarray(inputs["chk_index"], np.int64))
    got = kernel(**inputs)
    err = np.max(np.abs(got - exp)) / (np.max(np.abs(exp)) + 1e-30)
    print("Relative error:", err)


# revision 3
# speedup vs baseline: 1.0001x; 1.0001x over previous
"""Trainium2 Bass kernel v5 for NeuralSumProductModel (LDPC sum-product).

Contract: kernel(llr [512,8192] f32, var_index [24576] i32, chk_index
[24576] i32) -> [5, 512, 8192] f32, batch-sharded 8 ways (BC=64/core).

v5: all-gather dataflow (dma_scatter_add drains at ~100ns/desc DRAM RMW
— unusable), fp16 staging padded to 256B rows so every gather desc is
256B (the legal minimum), all contiguous/strided writes on HWDGE (no
gpsimd cost):
  - OUTR[2] fp16 [N_VAR, 128]: out rows (64 payload + 64 pad), ping-pong.
  - EXTR fp16 [E, 128]: check-major ext rows, written strided by HWDGE.
  - check unit (3072 edges): msg-gather out[var(e)] (1 desc/edge, 256B)
    -> msg = g - ext_prev (1 DVE op); Tanh -> t(f32); prefix/suffix
    products -> loo (signs ride along); clamp +-(1-1e-7); ext =
    Ln(1+loo) - Ln(1-loo); strided HWDGE write to EXTR.
  - var phase (4 chunks): var-gather each var's 3 ext rows (1 desc each)
    -> out = x + s0+s1+s2 -> strided HWDGE write to OUTR[next].
  - out_d: ACT transpose-copy + write, deferred into the next iteration.
  - iteration 0 needs no special case: OUTR[0] pre-initialized with x,
    ext_prev = 0 (memset).
  - Tanh/Ln activation tables batched pairwise (8 loads/iter).
"""

import os
import sys

import numpy as np

for _p in ("/opt/trn_rl_repo", "/root/.axon_site/_ro/trn_rl_repo"):
    if os.path.isdir(_p) and _p not in sys.path:
        sys.path.insert(0, _p)

N_VAR, N_CHK, DV, DC = 8192, 4096, 3, 6
E = N_VAR * DV                  # 24576
BATCH, N_ITER, N_CORES = 512, 5, 8
BC = BATCH // N_CORES           # 64 batch rows per core
NU = 8                          # check units per iteration
UE = E // NU                    # 3072 edges per unit
UJ = UE // 128                  # 24 gathered blocks per unit
CU = UJ * BC                    # 1536 payload cols per unit
CTU = N_CHK // 128 // NU        # 4 check tiles per unit
GCH = 1024                      # idxs per SWDGE call (ring cap)
VROW = 128                      # staging row width (64 payload + 64 pad)
NVC = 4                         # var-phase chunks
VCH = N_VAR // 128 // NVC       # 16 vars per partition per chunk

EPS = 1e-12
CLIP = float(np.float32(1.0) - np.float32(1e-7))

_CACHE = {}
_LAST_RESULTS = None


def _wrap(stream):
    """[n] int -> wrapped [128, n//16] int16 (16-partition wrap, x8 cores)."""
    st = np.asarray(stream, np.int16)
    n = st.shape[0]
    assert n % 16 == 0
    core = st.reshape(n // 16, 16).T
    return np.tile(core, (8, 1))


def _build_indices(vi, ci):
    """Host-side graph preprocessing.

    Check-major slot j = u*3072 + l*128 + p  (l = ct_loc*6 + e) maps to
    check c = (4u + l//6)*128 + p, edge e = l%6 of the by-check-sorted
    edge list. EXTR row == slot j. Var-gather output row i = n*128 + p
    (n global) covers (vt, s) = (n//3, n%3), var v = p*64 + vt.
    """
    order = np.argsort(ci, kind="stable")
    cm_var = vi[order].astype(np.int64)

    u = np.arange(NU)[:, None, None]
    l = np.arange(UJ)[None, :, None]
    p = np.arange(128)[None, None, :]
    c = (CTU * u + l // DC) * 128 + p
    cm_idx = c * DC + (l % DC)
    v_slot = cm_var[cm_idx].reshape(E)           # var id per slot j

    # j-positions of each var's 3 edges (sorted by j): pos[v*3+s]
    pos = np.argsort(v_slot, kind="stable")      # [E]

    n = np.arange(N_VAR // 128 * DV)[:, None]    # 192 rows per partition
    p2 = np.arange(128)[None, :]
    v = p2 * (N_VAR // 128) + n // DV
    vidx = pos[v * DV + n % DV]                  # [192, 128]
    vidx = vidx.reshape(-1)                      # i = n*128 + p order

    return _wrap(v_slot), _wrap(vidx), v_slot


def _build_bass():
    import concourse.tile as tile
    from concourse import bacc, mybir
    from contextlib import ExitStack

    dt = mybir.dt
    F32, F16, I16 = dt.float32, dt.float16, dt.int16
    ALU = mybir.AluOpType
    ACT = mybir.ActivationFunctionType

    nc = bacc.Bacc("TRN2", target_bir_lowering=False, debug=False,
                   num_swdge_queues=4)

    gidx_d = nc.dram_tensor("gidx", [128, E // 16], I16,
                            kind="ExternalInput").ap()
    vidx_d = nc.dram_tensor("vidx", [128, E // 16], I16,
                            kind="ExternalInput").ap()
    xv_d = nc.dram_tensor("xv", [N_VAR, BC], F16, kind="ExternalInput").ap()
    xg_d = nc.dram_tensor("xg", [E, BC], F16, kind="ExternalInput").ap()
    out_d = nc.dram_tensor("out", [N_ITER, BC, N_VAR], F32,
                           kind="ExternalOutput").ap()
    outr = [nc.dram_tensor(f"outr{i}", [N_VAR, VROW], F16,
                           kind="Internal").ap() for i in range(2)]
    extr = nc.dram_tensor("extr", [E, VROW], F16, kind="Internal").ap()

    with tile.TileContext(nc) as tc, ExitStack() as ctx:
        big = ctx.enter_context(tc.tile_pool(name="big", bufs=1))

        G = [big.tile([128, UJ * VROW], F16, tag=f"g{i}", name=f"G{i}")
             for i in range(3)]                        # [p,24*128] 6KB each
        GV = [big.tile([128, VCH * DV * VROW], F16, tag=f"gv{i}",
                       name=f"GV{i}") for i in range(3)]  # [p,48*128] 12KB
        xvt = big.tile([128, N_VAR // 2], F16, tag="xv")   # 8KB
        xgt = big.tile([128, E * BC // 128], F16, tag="xg")  # 24KB
        exts = big.tile([128, E * BC // 128], F16, tag="exts")  # 24KB
        T = [big.tile([128, CU], F32, tag=f"t{i}", name=f"T{i}")
             for i in range(2)]
        LOO = [big.tile([128, CU], F32, tag=f"loo{i}", name=f"LOO{i}")
               for i in range(2)]
        MA = [big.tile([128, CU], F16, tag=f"ma{i}", name=f"MA{i}")
              for i in range(2)]
        MB = [big.tile([128, CU], F16, tag=f"mb{i}", name=f"MB{i}")
              for i in range(2)]
        sA = big.tile([128, CTU * BC], F32, tag="sa")      # 1KB
        sB = big.tile([128, CTU * BC], F32, tag="sb")
        vtA = big.tile([128, VCH * BC], F16, tag="vta")    # 2KB
        vtB = big.tile([128, VCH * BC], F16, tag="vtb")    # 2KB
        OV = [big.tile([128, N_VAR // 2], F16, tag=f"ov{i}", name=f"OV{i}")
              for i in range(2)]                           # 8KB each
        ob = big.tile([128, N_VAR // 2], F32, tag="ob")    # 16KB
        bclip = big.tile([128, 1], F32, tag="bclip")
        beps = big.tile([128, 1], F32, tag="beps")
        gidx_t = big.tile([128, E // 16], I16, tag="gidx")
        vidx_t = big.tile([128, E // 16], I16, tag="vidx")

        nc.sync.dma_start(gidx_t[:], gidx_d[:])
        nc.sync.dma_start(vidx_t[:], vidx_d[:])
        nc.sync.dma_start(xvt[:].rearrange("p (n k) -> p n k", k=BC),
                          xv_d[:, :].rearrange("(p n) k -> p n k", p=128))
        nc.sync.dma_start(xgt[:].rearrange("p (n k) -> p n k", k=BC),
                          xg_d[:, :].rearrange("(n p) k -> p n k", p=128))
        nc.vector.memset(bclip[:], CLIP)
        nc.vector.memset(beps[:], 1.0 - CLIP)
        qc = [0]

        def nextq():
            qc[0] += 1
            return qc[0] % 4

        # OUTR[0] := x rows (strided: 64 payload cols of each 128-col row)
        outr_s = [o[:, :].rearrange("(p n) k -> p n k", p=128)
                  for o in outr]                       # [128, 64, 128]
        extr_s = extr[:, :].rearrange("(n p) k -> p n k", p=128)
        nc.sync.dma_start(outr_s[0][:, :, 0:BC],
                          xvt[:].rearrange("p (n k) -> p n k", k=BC))

        def gathers(it, u):
            src = outr[it % 2][:, :]
            for c in range(3):
                g0 = u * UE + c * GCH
                dsl = G[u % 3][:, c * 8 * VROW:(c + 1) * 8 * VROW]
                nc.gpsimd.dma_gather(
                    dsl.rearrange("p (n k) -> p n k", k=VROW), src,
                    gidx_t[:, g0 // 16:(g0 + GCH) // 16],
                    num_idxs=GCH, num_idxs_reg=GCH, elem_size=VROW,
                    queue_num=nextq())

        def msg(u):
            gv = G[u % 3][:].rearrange("p (n k) -> p n k", k=VROW)
            mbv = MB[u % 2][:].rearrange("p (n k) -> p n k", k=BC)
            epre = exts[:, u * CU:(u + 1) * CU].rearrange(
                "p (n k) -> p n k", k=BC)
            nc.vector.tensor_tensor(mbv, gv[:, :, 0:BC], epre,
                                    op=ALU.subtract)
            return MB[u % 2]

        def products(u):
            t6 = T[u % 2][:].rearrange("p (ct e b) -> p ct e b", ct=CTU, e=DC)
            l6 = LOO[u % 2][:].rearrange("p (ct e b) -> p ct e b",
                                         ct=CTU, e=DC)
            t = [t6[:, :, e, :] for e in range(DC)]
            l = [l6[:, :, e, :] for e in range(DC)]
            a = sA[:].rearrange("p (ct b) -> p ct b", ct=CTU)
            b = sB[:].rearrange("p (ct b) -> p ct b", ct=CTU)
            M = ALU.mult
            nc.vector.tensor_tensor(l[1], t[0], t[1], op=M)   # pre2
            nc.vector.tensor_tensor(l[2], l[1], t[2], op=M)   # pre3
            nc.vector.tensor_tensor(l[3], l[2], t[3], op=M)   # pre4
            nc.vector.tensor_tensor(l[5], l[3], t[4], op=M)   # pre5
            nc.vector.tensor_tensor(l[4], l[3], t[5], op=M)   # pre4*suf5
            nc.vector.tensor_tensor(a, t[4], t[5], op=M)      # suf4
            nc.vector.tensor_tensor(l[3], l[2], a, op=M)      # pre3*suf4
            nc.vector.tensor_tensor(b, t[3], a, op=M)         # suf3
            nc.vector.tensor_tensor(l[2], l[1], b, op=M)      # pre2*suf3
            nc.vector.tensor_tensor(a, t[2], b, op=M)         # suf2
            nc.vector.tensor_tensor(l[1], t[0], a, op=M)      # pre1*suf2
            nc.vector.tensor_tensor(l[0], t[1], a, op=M)      # suf1

        def clamp_ln_ext(it, u):
            nc.vector.tensor_scalar(T[u % 2][:], LOO[u % 2][:], CLIP, -CLIP,
                                    op0=ALU.min, op1=ALU.max)
            nc.scalar.activation(MA[u % 2][:], T[u % 2][:], ACT.Ln,
                                 bias=1.0, scale=1.0)
            nc.scalar.activation(MB[u % 2][:], T[u % 2][:], ACT.Ln,
                                 bias=1.0, scale=-1.0)
            nc.vector.tensor_tensor(exts[:, u * CU:(u + 1) * CU],
                                    MA[u % 2][:], MB[u % 2][:],
                                    op=ALU.subtract)
            # strided HWDGE write of this unit's ext rows to EXTR
            nc.sync.dma_start(
                extr_s[:, u * UJ:(u + 1) * UJ, 0:BC],
                exts[:, u * CU:(u + 1) * CU].rearrange(
                    "p (n k) -> p n k", k=BC))

        def var_gathers(c):
            for q in range(6):
                g0 = c * DV * VCH * 128 + q * GCH
                dsl = GV[c % 3][:, q * 8 * VROW:(q + 1) * 8 * VROW]
                nc.gpsimd.dma_gather(
                    dsl.rearrange("p (n k) -> p n k", k=VROW),
                    extr[:, :],
                    vidx_t[:, g0 // 16:(g0 + GCH) // 16],
                    num_idxs=GCH, num_idxs_reg=GCH, elem_size=VROW,
                    queue_num=nextq())

        def var_sums(it, c):
            g3 = GV[c % 3][:].rearrange("p (vt s k) -> p vt s k",
                                        s=DV, k=VROW)
            s0, s1, s2 = (g3[:, :, s, 0:BC] for s in range(DV))
            w = VCH * BC
            va = vtA[:].rearrange("p (n k) -> p n k", k=BC)
            vb = vtB[:].rearrange("p (n k) -> p n k", k=BC)
            ovw = OV[it % 2][:, c * w:(c + 1) * w].rearrange(
                "p (n k) -> p n k", k=BC)
            xvw = xvt[:, c * w:(c + 1) * w].rearrange("p (n k) -> p n k", k=BC)
            nc.vector.tensor_tensor(va, s0, s1, op=ALU.add)
            nc.vector.tensor_tensor(vb, va, s2, op=ALU.add)
            nc.vector.tensor_tensor(ovw, vb, xvw, op=ALU.add)
            # strided write of this chunk's out rows to OUTR[next]
            if it + 1 < N_ITER:
                nc.sync.dma_start(
                    outr_s[(it + 1) % 2][:, c * VCH:(c + 1) * VCH, 0:BC], ovw)

        def out_finish(src_it):
            nc.scalar.activation(
                ob[:].rearrange("p (b vt) -> p b vt", b=BC),
                OV[src_it % 2][:].rearrange("p (vt b) -> p b vt",
                                            vt=N_VAR // 128),
                ACT.Copy)
            nc.sync.dma_start(
                out_d[src_it].rearrange("b (p vt) -> p b vt", p=128),
                ob[:].rearrange("p (b vt) -> p b vt", b=BC))

        for it in range(N_ITER):
            if it > 0:
                gathers(it, 0)
                gathers(it, 1)
                gathers(it, 2)
            for k in range(NU // 2):
                a, b2 = 2 * k, 2 * k + 1
                src_a = msg(a)[:] if it > 0 \
                    else xgt[:, a * CU:(a + 1) * CU]
                nc.scalar.activation(T[a % 2][:], src_a, ACT.Tanh, scale=0.5)
                if it > 0 and 2 * k + 3 < NU:
                    gathers(it, 2 * k + 3)
                src_b = msg(b2)[:] if it > 0 \
                    else xgt[:, b2 * CU:(b2 + 1) * CU]
                nc.scalar.activation(T[b2 % 2][:], src_b, ACT.Tanh, scale=0.5)
                if it > 0 and 2 * k + 4 < NU:
                    gathers(it, 2 * k + 4)
                products(a)
                clamp_ln_ext(it, a)
                products(b2)
                clamp_ln_ext(it, b2)
                if k == 0 and it > 0:
                    out_finish(it - 1)
            # var phase: gather each var's 3 ext rows, out = x + sums
            var_gathers(0)
            var_gathers(1)
            var_gathers(2)
            var_sums(it, 0)
            var_gathers(3)
            var_sums(it, 1)
            var_sums(it, 2)
            var_sums(it, 3)
        out_finish(N_ITER - 1)

    nc.compile()
    return nc


def _numpy_fallback(llr, vi, ci):
    x = llr.T.astype(np.float32)
    scattered = x[vi]
    ext = np.zeros_like(scattered)
    outs = []
    for _ in range(N_ITER):
        vsum = np.zeros((N_VAR, x.shape[1]), np.float32)
        np.add.at(vsum, vi, ext)
        msg = (vsum[vi] - ext) + scattered
        t = np.tanh(msg * 0.5)
        la = np.log(np.abs(t) + EPS)
        sg = np.sign(t)
        cs = np.zeros((N_CHK, x.shape[1]), np.float32)
        np.add.at(cs, ci, la)
        cpr = np.ones((N_CHK, x.shape[1]), np.float32)
        np.multiply.at(cpr, ci, sg)
        loo = np.exp(cs[ci] - la) * (cpr[ci] * sg)
        loo = np.clip(loo, -CLIP, CLIP)
        ext = 2.0 * np.arctanh(loo)
        vs2 = np.zeros((N_VAR, x.shape[1]), np.float32)
        np.add.at(vs2, vi, ext)
        outs.append((vs2 + x).T)
    return np.stack(outs)


def kernel(llr, var_index, chk_index):
    llr = np.asarray(llr, np.float32)
    vi = np.asarray(var_index, np.int64).ravel()
    ci = np.asarray(chk_index, np.int64).ravel()
    assert llr.shape == (BATCH, N_VAR) and vi.shape == (E,) and ci.shape == (E,)

    regular = (np.array_equal(np.bincount(vi, minlength=N_VAR),
                              np.full(N_VAR, DV))
               and np.array_equal(np.bincount(ci, minlength=N_CHK),
                                  np.full(N_CHK, DC)))
    if not regular:
        return _numpy_fallback(llr, vi, ci).astype(np.float32)

    key = ("v5", hash(vi.tobytes()), hash(ci.tobytes()))
    if key not in _CACHE:
        gidx, vidx, v_slot = _build_indices(vi, ci)
        nc = _build_bass()
        _CACHE[key] = (nc, gidx, vidx, v_slot)
    nc, gidx, vidx, v_slot = _CACHE[key]

    from concourse.bass_utils import run_bass_kernel_spmd
    llr16 = llr.astype(np.float16)
    in_maps = []
    for c in range(N_CORES):
        xc = llr16[c * BC:(c + 1) * BC, :]
        m = {
            "gidx": np.ascontiguousarray(gidx),
            "vidx": np.ascontiguousarray(vidx),
            "xv": np.ascontiguousarray(xc.T),
            "xg": np.ascontiguousarray(xc.T[v_slot]),
        }
        in_maps.append(m)
    trace = os.environ.get("BASS_KERNEL_TRACE", "0") == "1"
    res = run_bass_kernel_spmd(nc, in_maps, list(range(N_CORES)), trace=trace)
    global _LAST_RESULTS
    _LAST_RESULTS = res
    out = np.concatenate([res.results[c]["out"] for c in range(N_CORES)],
                         axis=1)
    return np.ascontiguousarray(out, dtype=np.float32)


if __name__ == "__main__":
    sys.path.insert(0, os.path.dirname(os.path.abspath(__file__)))
    import reference
    inputs = {k: np.asarray(v) for k, v in reference.setup_inputs().items()}
    exp = _numpy_fallback(np.asarray(inputs["llr"], np.float32),
                          np.asarray(inputs["var_index"], np.int64),
                          np.asarray(inputs["chk_index"], np.int64))
    got = kernel(**inputs)
    err = np.max(np.abs(got - exp)) / (np.max(np.abs(exp)) + 1e-30)
    print("Relative error:", err)
